# revision 3
# baseline (speedup 1.0000x reference)
"""Causal segment-masked depthwise conv (K=5) + pointwise conv, 8-core SPMD.

Strategy (v2, bf16):
  Host: pack each batch row's segments into a global stream with 4 zeros
  before each segment (plain causal conv on the stream == per-segment
  left-zero-padded conv), split the stream evenly across 8 cores with a
  4-element halo, pre-transpose to [C, stream] and cast to bf16.
  Biases fold out of the device: out = Wpw.conv + (Wpw@b_dw + b_pw); the
  constant rides the ACT out-copies' per-partition bias operand.
  Device per core (stream width 4160 = 4 superblocks of 1024 + 64 tail):
    dw conv split by tap across engines:
      PE   : chunk0 all 5 taps, chunk1 taps {1,3,4} (diag-stationary bf16
             matmuls, PSUM f32, ACT copy -> bf16), full tail.
      DVE  : chunk1 taps {0,2} + chunks 2,3 all taps as tensor_scalar
             products (bf16 4x mode) + tensor_tensor adds (bf16 2x mode);
             odd-shift taps read a 1-element-shifted dup slab built by
             GPSIMD so every DVE stream stays 4B-aligned.
    pw: 16 bf16 matmuls per 512-block, j-major with rhs ordered by
        availability (dwt0, a2, a1-merged, a3), 4 concurrent PSUM banks,
        ACT copies PSUM -> bf16 out tile adding the folded bias.
  Host transposes back during gather and applies a sparse general-case
  correction for exotic segment overlap patterns (empty for contiguous
  partitions).
"""

import sys

sys.path.insert(0, "/opt/trn_rl_repo")

import numpy as np
import ml_dtypes

BF16 = ml_dtypes.bfloat16

B, L, C, K, S = 8, 4096, 512, 5, 8
NCORES = 8
CCH = C // 128          # 4 channel chunks
NSB = 4                 # superblocks of 1024
SBW = NSB * 1024        # 4096
TAILW = 64              # tail block width
TOTW = SBW + TAILW      # 4160 per-core processed stream width
XSW = 1040              # packed slab piece width (1024 + 4 halo + pad)
XQW = 72                # tail slab width (64 + 4 halo + pad)
DUPW = 1032             # dup slab width
PE_T1 = (1, 3, 4)       # chunk1 taps on PE
DVE_T1 = (0, 2)         # chunk1 taps on DVE (even shifts: no dup needed)

_cached = {}


def _build_nc():
    import concourse.mybir as mybir
    from concourse import bacc
    from concourse.tile import TileContext

    f32 = mybir.dt.float32
    bf16 = mybir.dt.bfloat16

    nc = bacc.Bacc(num_swdge_queues=2)
    xs_d = nc.declare_dram_parameter("xs", [NSB, 128, CCH, XSW], bf16, isOutput=False)
    xq_d = nc.declare_dram_parameter("xq", [128, CCH, XQW], bf16, isOutput=False)
    # cst: [0:20]=wdiag f32, [20:24]=bout f32
    cst_d = nc.declare_dram_parameter("cst", [128, CCH * K + CCH], f32, isOutput=False)
    ident_d = nc.declare_dram_parameter("ident", [128, 128], bf16, isOutput=False)
    wpwt_d = nc.declare_dram_parameter("wpwt", [128, CCH, CCH, 128], bf16, isOutput=False)
    out_d = nc.declare_dram_parameter("out", [128, CCH, TOTW], bf16, isOutput=True)

    with TileContext(nc) as tc:
        with (
            tc.tile_pool(name="consts", bufs=1) as cpool,
            tc.tile_pool(name="xc0", bufs=2) as x0_pool,
            tc.tile_pool(name="xc1", bufs=2) as x1_pool,
            tc.tile_pool(name="xc2", bufs=2) as x2_pool,
            tc.tile_pool(name="xc3", bufs=2) as x3_pool,
            tc.tile_pool(name="xd2", bufs=2) as d2_pool,
            tc.tile_pool(name="xd3", bufs=2) as d3_pool,
            tc.tile_pool(name="acc1", bufs=2) as a1_pool,
            tc.tile_pool(name="acc2", bufs=2) as a2_pool,
            tc.tile_pool(name="acc3", bufs=2) as a3_pool,
            tc.tile_pool(name="tprod", bufs=2) as tp_pool,
            tc.tile_pool(name="dwt0", bufs=3) as dwt0_pool,
            tc.tile_pool(name="dwp1", bufs=3) as dwp1_pool,
            tc.tile_pool(name="outsb", bufs=3) as ob_pool,
            tc.tile_pool(name="dps0", bufs=2, space="PSUM") as dps0,
            tc.tile_pool(name="dps1", bufs=2, space="PSUM") as dps1,
            tc.tile_pool(name="pwps", bufs=4, space="PSUM") as pwps,
        ):
            # ---- consts (sync ring, first) ----
            cst = cpool.tile([128, CCH * K + CCH], f32)
            nc.sync.dma_start(out=cst[:], in_=cst_d[:])
            wdiag = cst[:, 0 : CCH * K]
            bout = cst[:, CCH * K : CCH * K + CCH]
            ident = cpool.tile([128, 128], bf16)
            nc.sync.dma_start(out=ident[:], in_=ident_d[:])
            xq = cpool.tile([128, CCH, XQW], bf16)
            nc.sync.dma_start(out=xq[:], in_=xq_d[:])

            # warm-up fodder: zero tile, no DMA dependency (Pool memset)
            warmz = cpool.tile([128, 512], bf16)
            nc.gpsimd.memset(warmz[:], 0.0)

            xt = {}

            def load_sync(pool, sb, c, tag):
                t = pool.tile([128, XSW], bf16, tag=tag, name=f"{tag}_{sb}")
                nc.sync.dma_start(out=t[:], in_=xs_d[sb, :, c, :])
                xt[(sb, c)] = t

            def load_pool(pool, sb, c, tag):
                t = pool.tile([128, XSW], bf16, tag=tag, name=f"{tag}_{sb}")
                nc.gpsimd.dma_start(out=t[:], in_=xs_d[sb, :, c, :])
                xt[(sb, c)] = t

            # PE chunks via the sync ring (in front of wpwt), DVE chunks via
            # the two SWDGE queues with the dup copy right behind each load.
            load_sync(x0_pool, 0, 0, "x0")
            load_sync(x1_pool, 0, 1, "x1")
            wpwt = cpool.tile([128, CCH, CCH, 128], bf16)
            nc.sync.dma_start(out=wpwt[:], in_=wpwt_d[:])
            for sb in range(1, NSB):
                load_sync(x0_pool, sb, 0, "x0")
                load_sync(x1_pool, sb, 1, "x1")

            dup = {}

            def make_dup(pool, sb, c, tag):
                t = pool.tile([128, DUPW], bf16, tag=tag, name=f"{tag}_{sb}")
                nc.gpsimd.tensor_copy(t[:], xt[(sb, c)][:, 1 : 1 + DUPW])
                dup[(sb, c)] = t

            for sb in range(NSB):
                load_pool(x2_pool, sb, 2, "x2")
                load_pool(x3_pool, sb, 3, "x3")
                make_dup(d2_pool, sb, 2, "d2")
                make_dup(d3_pool, sb, 3, "d3")

            # PE warm-up: lift the HAM clock gate while DMAs land
            for wi in range(6):
                wps = pwps.tile([128, 512], f32, tag="pwps", name=f"warm{wi}")
                nc.tensor.matmul(
                    wps[:], lhsT=warmz[:, 0:128], rhs=warmz[:], start=True, stop=True
                )

            # diag tiles built on device: ACT does chunks 0,1; DVE chunks 2,3
            diag = cpool.tile([128, CCH * K, 128], bf16)
            for u in range(CCH * K):
                sl = diag[:, u, :]
                wc = wdiag[:, u : u + 1]
                if u < 2 * K:
                    nc.scalar.mul(sl, ident[:], wc)
                else:
                    nc.vector.tensor_scalar_mul(sl, ident[:], wc)

            # ---- DVE helpers ----
            def dve_full_chunk(sb, c, pool):
                """acc = sum_k w_k * x[k:k+1024] for chunk c (all 5 taps)."""
                A = xt[(sb, c)]
                Bs = dup[(sb, c)]
                acc = pool.tile([128, 1024], bf16, tag=f"a{c}", name=f"a{c}_{sb}")
                nc.vector.tensor_scalar_mul(
                    acc[:], A[:, 0:1024], wdiag[:, c * K : c * K + 1]
                )
                for k in range(1, K):
                    tp = tp_pool.tile(
                        [128, 1024], bf16, tag="tp", name=f"tp{c}_{sb}_{k}"
                    )
                    if k % 2 == 1:
                        src = Bs[:, k - 1 : k - 1 + 1024]
                    else:
                        src = A[:, k : k + 1024]
                    nc.vector.tensor_scalar_mul(
                        tp[:], src, wdiag[:, c * K + k : c * K + k + 1]
                    )
                    nc.vector.tensor_add(acc[:], acc[:], tp[:])
                return acc

            def dve_c1_partial(sb):
                """chunk1 DVE taps {0,2} (both even shifts, no dup needed)."""
                A = xt[(sb, 1)]
                acc = a1_pool.tile([128, 1024], bf16, tag="a1", name=f"a1_{sb}")
                k0, k1 = DVE_T1
                nc.vector.tensor_scalar_mul(
                    acc[:], A[:, k0 : k0 + 1024], wdiag[:, K + k0 : K + k0 + 1]
                )
                tp = tp_pool.tile([128, 1024], bf16, tag="tp", name=f"tp1_{sb}")
                nc.vector.tensor_scalar_mul(
                    tp[:], A[:, k1 : k1 + 1024], wdiag[:, K + k1 : K + k1 + 1]
                )
                nc.vector.tensor_add(acc[:], acc[:], tp[:])
                return acc

            # ---- PE dw conv for one 512-block of one chunk ----
            def pe_conv(sb, b, c, taps, psum_pool, out_pool, tag):
                ps = psum_pool.tile(
                    [128, 512], f32, tag=f"dps{c}", name=f"ps{c}_{sb}_{b}"
                )
                x = xt[(sb, c)]
                off = b * 512
                taps = list(taps)
                for i, k in enumerate(taps):
                    nc.tensor.matmul(
                        ps[:],
                        lhsT=diag[:, c * K + k, :],
                        rhs=x[:, off + k : off + k + 512],
                        start=(i == 0),
                        stop=(i == len(taps) - 1),
                    )
                dt_ = out_pool.tile([128, 512], bf16, tag=tag, name=f"{tag}_{sb}_{b}")
                nc.scalar.copy(dt_[:], ps[:])
                return dt_

            # ---- pointwise for one 512-block ----
            def pointwise(lb, rhs_of, store_eng):
                pos = [
                    pwps.tile([128, 512], f32, tag="pwps", name=f"po{dch}_{lb}")
                    for dch in range(CCH)
                ]
                for jj, j in enumerate((0, 2, 1, 3)):
                    for dch in range(CCH):
                        nc.tensor.matmul(
                            pos[dch][:],
                            lhsT=wpwt[:, j, dch, :],
                            rhs=rhs_of[j],
                            start=(jj == 0),
                            stop=(jj == CCH - 1),
                        )
                ob = ob_pool.tile([128, CCH, 512], bf16, tag="ob", name=f"ob_{lb}")
                for dch in range(CCH):
                    nc.scalar.add(ob[:, dch, :], pos[dch][:], bout[:, dch : dch + 1])
                off = lb * 512
                store_eng.dma_start(out=out_d[:, :, off : off + 512], in_=ob[:])

            # ---- tail block (cols 4096..4159), all 4 chunks on PE ----
            def tail_block():
                dwq = []
                for c in range(CCH):
                    ps = (dps0 if c % 2 == 0 else dps1).tile(
                        [128, TAILW], f32, tag=f"dps{c % 2}", name=f"psq{c}"
                    )
                    for k in range(K):
                        nc.tensor.matmul(
                            ps[:],
                            lhsT=diag[:, c * K + k, :],
                            rhs=xq[:, c, k : k + TAILW],
                            start=(k == 0),
                            stop=(k == K - 1),
                        )
                    dt_ = (dwt0_pool if c % 2 == 0 else dwp1_pool).tile(
                        [128, TAILW],
                        bf16,
                        tag=("dwt0" if c % 2 == 0 else "dwp1"),
                        name=f"dwq{c}",
                    )
                    nc.scalar.copy(dt_[:], ps[:])
                    dwq.append(dt_)
                pos = [
                    pwps.tile([128, TAILW], f32, tag="pwps", name=f"poq{dch}")
                    for dch in range(CCH)
                ]
                for j in range(CCH):
                    for dch in range(CCH):
                        nc.tensor.matmul(
                            pos[dch][:],
                            lhsT=wpwt[:, j, dch, :],
                            rhs=dwq[j][:],
                            start=(j == 0),
                            stop=(j == CCH - 1),
                        )
                ob = ob_pool.tile([128, CCH, TAILW], bf16, tag="obq", name="ob_q")
                for dch in range(CCH):
                    nc.scalar.add(ob[:, dch, :], pos[dch][:], bout[:, dch : dch + 1])
                nc.sync.dma_start(out=out_d[:, :, SBW : SBW + TAILW], in_=ob[:])

            # ---- main pipeline ----
            tail_block()
            for sb in range(NSB):
                a1 = dve_c1_partial(sb)
                a2 = dve_full_chunk(sb, 2, a2_pool)

                dwt0 = [pe_conv(sb, b, 0, range(K), dps0, dwt0_pool, "dwt0") for b in range(2)]
                dwp1 = [pe_conv(sb, b, 1, PE_T1, dps1, dwp1_pool, "dwp1") for b in range(2)]

                # merge chunk1: a1 += PE partial (bf16 TT, 2x mode)
                for b in range(2):
                    nc.vector.tensor_add(
                        a1[:, b * 512 : (b + 1) * 512],
                        a1[:, b * 512 : (b + 1) * 512],
                        dwp1[b][:],
                    )

                a3 = dve_full_chunk(sb, 3, a3_pool)

                for b in range(2):
                    lb = sb * 2 + b
                    o = b * 512
                    rhs_of = {
                        0: dwt0[b][:],
                        1: a1[:, o : o + 512],
                        2: a2[:, o : o + 512],
                        3: a3[:, o : o + 512],
                    }
                    st = (nc.sync, nc.scalar)[lb % 2] if lb < 6 else nc.gpsimd
                    pointwise(lb, rhs_of, st)

    nc.finalize()
    return nc


def _get_nc():
    if "nc" not in _cached:
        _cached["nc"] = _build_nc()
    return _cached["nc"]


def _analyze(segment_boundaries):
    starts = segment_boundaries[..., 0].astype(np.int64)  # [B,S]
    ends = segment_boundaries[..., 1].astype(np.int64)
    pos = np.arange(L)
    in_seg = (pos[None, None, :] >= starts[..., None]) & (
        pos[None, None, :] < ends[..., None]
    )  # [B,S,L]
    covered = in_seg.any(axis=1)
    seg_id = np.where(covered, in_seg.argmax(axis=1), -1)  # [B,L]
    return covered, seg_id


def kernel(x, segment_boundaries, w_dw, b_dw, w_pw, b_pw):
    from concourse.bass_utils import run_bass_kernel_spmd

    x = np.asarray(x, dtype=np.float32)
    sb = np.asarray(segment_boundaries)
    w_dw = np.asarray(w_dw, dtype=np.float32)
    b_dw = np.asarray(b_dw, dtype=np.float32)
    w_pw = np.asarray(w_pw, dtype=np.float32)
    b_pw = np.asarray(b_pw, dtype=np.float32)

    covered, seg_id = _analyze(sb)

    # ---- run decomposition + stream build ----
    pieces = []
    src_b_parts = []
    src_l_parts = []
    run_start_of = np.full((B, L), -1, np.int64)
    for b in range(B):
        sid = seg_id[b]
        change = np.nonzero(np.diff(sid) != 0)[0] + 1
        bounds = np.concatenate([[0], change, [L]])
        for s, e in zip(bounds[:-1], bounds[1:]):
            if sid[s] < 0:
                continue
            run_start_of[b, s:e] = s
            pieces.append(np.zeros((4, C), np.float32))
            src_b_parts.append(np.full(4, -1, np.int64))
            src_l_parts.append(np.full(4, -1, np.int64))
            pieces.append(x[b, s:e])
            src_b_parts.append(np.full(e - s, b, np.int64))
            src_l_parts.append(np.arange(s, e, dtype=np.int64))
    if pieces:
        stream = np.concatenate(pieces, axis=0)
        src_b = np.concatenate(src_b_parts)
        src_l = np.concatenate(src_l_parts)
    else:
        stream = np.zeros((0, C), np.float32)
        src_b = np.zeros(0, np.int64)
        src_l = np.zeros(0, np.int64)
    T = stream.shape[0]
    Q = -(-T // NCORES) if T else 1
    assert Q <= TOTW, f"stream quota {Q} too large"

    # ---- shared per-core inputs ----
    wdiag = np.ascontiguousarray(
        w_dw.reshape(CCH, 128, K).transpose(1, 0, 2).reshape(128, CCH * K)
    ).astype(np.float32)
    bias_out = w_pw @ b_dw + b_pw
    boutr = np.ascontiguousarray(bias_out.reshape(CCH, 128).T).astype(np.float32)
    cst = np.concatenate([wdiag, boutr], axis=1)
    ident = np.eye(128, dtype=np.float32).astype(BF16)
    wpwt = np.ascontiguousarray(
        w_pw.reshape(CCH, 128, CCH, 128).transpose(3, 2, 0, 1)
    ).astype(BF16)

    SLAB_W = 4 + TOTW + 16
    in_maps = []
    spans = []
    for i in range(NCORES):
        lo, hi = i * Q, min((i + 1) * Q, T)
        lo = min(lo, T)
        spans.append((lo, hi))
        buf = np.zeros((SLAB_W, C), np.float32)
        if hi > lo:
            hlo = max(0, lo - 4)
            buf[4 - (lo - hlo) : 4 + (hi - lo)] = stream[hlo:hi]
        slabT = np.ascontiguousarray(buf.T).astype(BF16)  # [C, SLAB_W]
        slabT = slabT.reshape(CCH, 128, SLAB_W)
        xs = np.zeros((NSB, 128, CCH, XSW), BF16)
        for sbi in range(NSB):
            xs[sbi, :, :, :1032] = slabT[
                :, :, sbi * 1024 : sbi * 1024 + 1032
            ].transpose(1, 0, 2)
        xq = np.zeros((128, CCH, XQW), BF16)
        xq[:, :, : TAILW + 4] = slabT[:, :, SBW : SBW + TAILW + 4].transpose(1, 0, 2)
        in_maps.append(
            {"xs": xs, "xq": xq, "cst": cst, "ident": ident, "wpwt": wpwt}
        )

    nc = _get_nc()
    res = run_bass_kernel_spmd(nc, in_maps, list(range(NCORES)))

    # ---- gather (device out is [128, CCH, TOTW] bf16) ----
    so_out = np.zeros((T, C), np.float32)
    for i, (lo, hi) in enumerate(spans):
        if hi > lo:
            # [p, ch, t] -> [t, ch*128+p]
            full = (
                np.asarray(res.results[i]["out"])
                .astype(np.float32)
                .transpose(2, 1, 0)
                .reshape(TOTW, C)
            )
            so_out[lo:hi] = full[: hi - lo]
    out = np.zeros((B, L, C), np.float32)
    mask = src_l >= 0
    out[src_b[mask], src_l[mask]] = so_out[mask]

    # ---- general-case sparse correction (pairwise mask vs run mask) ----
    need = []
    for d in range(1, K):
        m_ref = np.zeros((B, L), bool)
        m_ref[:, d:] = covered[:, d:] & (seg_id[:, d:] == seg_id[:, :-d])
        m_run = covered & (np.arange(L)[None, :] - run_start_of >= d)
        diff = m_ref.astype(np.int8) - m_run.astype(np.int8)
        if np.any(diff):
            bs, ls = np.nonzero(diff)
            need.append((d, bs, ls, diff[bs, ls].astype(np.float32)))
    if need:
        for d, bs, ls, sgn in need:
            xv_ = x[bs, ls - d, :]
            delta_dw = xv_ * w_dw[None, :, K - 1 - d] * sgn[:, None]
            out[bs, ls, :] += delta_dw @ w_pw.T

    return out


# revision 4
# speedup vs baseline: 1.2376x; 1.2376x over previous
"""Causal segment-masked depthwise conv (K=5) + pointwise conv, 8-core SPMD.

Strategy (v2.1, bf16):
  Host: pack each batch row's segments into a global stream with 4 zeros
  before each segment (plain causal conv on the stream == per-segment
  left-zero-padded conv), split the stream evenly across 8 cores with a
  4-element halo, pre-transpose to [C, stream] and cast to bf16.
  Biases fold out of the device: out = Wpw.conv + (Wpw@b_dw + b_pw); the
  constant rides the ACT out-copies' per-partition bias operand.
  Device per core (stream width 4160 = 4 superblocks of 1024 + 64 tail):
    dw conv:
      PE   : chunks 0,1 all taps (diag-stationary bf16 matmuls shipped
             prebuilt from host, PSUM f32, ACT copy -> bf16), full tail.
      DVE  : chunks 2,3 all taps as tensor_scalar products (bf16 fast
             mode) + tensor_tensor adds (bf16 2x mode); odd-shift taps
             read a host-packed 1-element-shifted dup slab so every DVE
             stream stays 4B-aligned.
    pw: 16 bf16 matmuls per 512-block, j-major with the DVE-produced rhs
        (j=2,3) last, 4 concurrent PSUM banks, ACT copies PSUM -> bf16
        out tile adding the folded bias.
  Host transposes back during gather and applies a sparse general-case
  correction for exotic segment overlap patterns (empty for contiguous
  partitions).
"""

import sys

sys.path.insert(0, "/opt/trn_rl_repo")

import numpy as np
import ml_dtypes

BF16 = ml_dtypes.bfloat16

B, L, C, K, S = 8, 4096, 512, 5, 8
NCORES = 8
CCH = C // 128          # 4 channel chunks
NSB = 4                 # superblocks of 1024
SBW = NSB * 1024        # 4096
TAILW = 64              # tail block width
TOTW = SBW + TAILW      # 4160 per-core processed stream width
XSW = 1040              # packed slab piece width (1024 + 4 halo + pad)
XQW = 72                # tail slab width (64 + 4 halo + pad)

_cached = {}


def _build_nc():
    import concourse.mybir as mybir
    from concourse import bacc
    from concourse.tile import TileContext

    f32 = mybir.dt.float32
    bf16 = mybir.dt.bfloat16

    nc = bacc.Bacc(num_swdge_queues=2)
    # planes: 0,1 = chunks 0,1 ; 2,3 = chunks 2,3 ; 4,5 = dup(2), dup(3)
    xs_d = nc.declare_dram_parameter("xs", [NSB, 128, 6, XSW], bf16, isOutput=False)
    xq_d = nc.declare_dram_parameter("xq", [128, CCH, XQW], bf16, isOutput=False)
    # cst: [0:20]=wdiag f32, [20:24]=bout f32
    cst_d = nc.declare_dram_parameter("cst", [128, CCH * K + CCH], f32, isOutput=False)
    diag_d = nc.declare_dram_parameter("diag", [128, CCH * K, 128], bf16, isOutput=False)
    wpwt_d = nc.declare_dram_parameter("wpwt", [128, CCH, CCH, 128], bf16, isOutput=False)
    out_d = nc.declare_dram_parameter("out", [128, CCH, TOTW], bf16, isOutput=True)

    with TileContext(nc) as tc:
        with (
            tc.tile_pool(name="consts", bufs=1) as cpool,
            tc.tile_pool(name="xc0", bufs=2) as x0_pool,
            tc.tile_pool(name="xc1", bufs=2) as x1_pool,
            tc.tile_pool(name="xc2", bufs=2) as x2_pool,
            tc.tile_pool(name="xc3", bufs=2) as x3_pool,
            tc.tile_pool(name="xd2", bufs=2) as d2_pool,
            tc.tile_pool(name="xd3", bufs=2) as d3_pool,
            tc.tile_pool(name="acc2", bufs=2) as a2_pool,
            tc.tile_pool(name="acc3", bufs=2) as a3_pool,
            tc.tile_pool(name="tprod", bufs=2) as tp_pool,
            tc.tile_pool(name="dwt0", bufs=3) as dwt0_pool,
            tc.tile_pool(name="dwt1", bufs=3) as dwt1_pool,
            tc.tile_pool(name="outsb", bufs=3) as ob_pool,
            tc.tile_pool(name="dps0", bufs=2, space="PSUM") as dps0,
            tc.tile_pool(name="dps1", bufs=2, space="PSUM") as dps1,
            tc.tile_pool(name="pwps", bufs=4, space="PSUM") as pwps,
        ):
            # ---- consts ----
            cst = cpool.tile([128, CCH * K + CCH], f32)
            nc.sync.dma_start(out=cst[:], in_=cst_d[:])
            wdiag = cst[:, 0 : CCH * K]
            bout = cst[:, CCH * K : CCH * K + CCH]
            xq = cpool.tile([128, CCH, XQW], bf16)
            nc.sync.dma_start(out=xq[:], in_=xq_d[:])

            # warm-up fodder: zero tile, no DMA dependency (Pool memset)
            warmz = cpool.tile([128, 512], bf16)
            nc.gpsimd.memset(warmz[:], 0.0)

            xt = {}

            def load(eng, pool, sb, plane, tag):
                t = pool.tile([128, XSW], bf16, tag=tag, name=f"{tag}_{sb}")
                eng.dma_start(out=t[:], in_=xs_d[sb, :, plane, :])
                xt[(sb, tag)] = t

            # sync ring: consts + PE chunks; SWDGE queues: DVE chunks + dups
            load(nc.sync, x0_pool, 0, 0, "x0")
            load(nc.sync, x1_pool, 0, 1, "x1")
            diag = cpool.tile([128, CCH * K, 128], bf16)
            nc.sync.dma_start(out=diag[:], in_=diag_d[:])
            wpwt = cpool.tile([128, CCH, CCH, 128], bf16)
            nc.sync.dma_start(out=wpwt[:], in_=wpwt_d[:])
            for sb in range(1, NSB):
                load(nc.sync, x0_pool, sb, 0, "x0")
                load(nc.sync, x1_pool, sb, 1, "x1")
            for sb in range(NSB):
                load(nc.gpsimd, x2_pool, sb, 2, "x2")
                load(nc.gpsimd, d2_pool, sb, 4, "d2")
                load(nc.gpsimd, x3_pool, sb, 3, "x3")
                load(nc.gpsimd, d3_pool, sb, 5, "d3")

            # PE warm-up: lift the HAM clock gate while DMAs land
            for wi in range(8):
                wps = pwps.tile([128, 512], f32, tag="pwps", name=f"warm{wi}")
                nc.tensor.matmul(
                    wps[:], lhsT=warmz[:, 0:128], rhs=warmz[:], start=True, stop=True
                )

            # ---- DVE dw conv for one chunk over one superblock ----
            def dve_chunk(sb, c, pool):
                A = xt[(sb, f"x{c}")]
                Bs = xt[(sb, f"d{c}")]
                acc = pool.tile([128, 1024], bf16, tag=f"a{c}", name=f"a{c}_{sb}")
                nc.vector.tensor_scalar_mul(
                    acc[:], A[:, 0:1024], wdiag[:, c * K : c * K + 1]
                )
                for k in range(1, K):
                    tp = tp_pool.tile(
                        [128, 1024], bf16, tag="tp", name=f"tp{c}_{sb}_{k}"
                    )
                    src = Bs[:, k - 1 : k - 1 + 1024] if k % 2 else A[:, k : k + 1024]
                    nc.vector.tensor_scalar_mul(
                        tp[:], src, wdiag[:, c * K + k : c * K + k + 1]
                    )
                    nc.vector.tensor_add(acc[:], acc[:], tp[:])
                return acc

            # ---- PE dw conv for one 512-block of one chunk ----
            def pe_conv(sb, b, c):
                pool = dps0 if c == 0 else dps1
                opool = dwt0_pool if c == 0 else dwt1_pool
                ps = pool.tile([128, 512], f32, tag=f"dps{c}", name=f"ps{c}_{sb}_{b}")
                x = xt[(sb, f"x{c}")]
                off = b * 512
                for k in range(K):
                    nc.tensor.matmul(
                        ps[:],
                        lhsT=diag[:, c * K + k, :],
                        rhs=x[:, off + k : off + k + 512],
                        start=(k == 0),
                        stop=(k == K - 1),
                    )
                dt_ = opool.tile(
                    [128, 512], bf16, tag=f"dwt{c}", name=f"dwt{c}_{sb}_{b}"
                )
                nc.scalar.copy(dt_[:], ps[:])
                return dt_

            # ---- pointwise for one 512-block ----
            def pointwise(lb, rhs_of, store_eng):
                pos = [
                    pwps.tile([128, 512], f32, tag="pwps", name=f"po{dch}_{lb}")
                    for dch in range(CCH)
                ]
                for jj in range(CCH):  # j = jj: 0,1 (ACT copies), 2,3 (DVE) last
                    for dch in range(CCH):
                        nc.tensor.matmul(
                            pos[dch][:],
                            lhsT=wpwt[:, jj, dch, :],
                            rhs=rhs_of[jj],
                            start=(jj == 0),
                            stop=(jj == CCH - 1),
                        )
                ob = ob_pool.tile([128, CCH, 512], bf16, tag="ob", name=f"ob_{lb}")
                for dch in range(CCH):
                    nc.scalar.add(ob[:, dch, :], pos[dch][:], bout[:, dch : dch + 1])
                off = lb * 512
                store_eng.dma_start(out=out_d[:, :, off : off + 512], in_=ob[:])

            # ---- tail block (cols 4096..4159), all 4 chunks on PE ----
            def tail_block():
                dwq = []
                for c in range(CCH):
                    ps = (dps0 if c % 2 == 0 else dps1).tile(
                        [128, TAILW], f32, tag=f"dps{c % 2}", name=f"psq{c}"
                    )
                    for k in range(K):
                        nc.tensor.matmul(
                            ps[:],
                            lhsT=diag[:, c * K + k, :],
                            rhs=xq[:, c, k : k + TAILW],
                            start=(k == 0),
                            stop=(k == K - 1),
                        )
                    dt_ = (dwt0_pool if c % 2 == 0 else dwt1_pool).tile(
                        [128, TAILW],
                        bf16,
                        tag=f"dwt{c % 2}",
                        name=f"dwq{c}",
                    )
                    nc.scalar.copy(dt_[:], ps[:])
                    dwq.append(dt_)
                pos = [
                    pwps.tile([128, TAILW], f32, tag="pwps", name=f"poq{dch}")
                    for dch in range(CCH)
                ]
                for j in range(CCH):
                    for dch in range(CCH):
                        nc.tensor.matmul(
                            pos[dch][:],
                            lhsT=wpwt[:, j, dch, :],
                            rhs=dwq[j][:],
                            start=(j == 0),
                            stop=(j == CCH - 1),
                        )
                ob = ob_pool.tile([128, CCH, TAILW], bf16, tag="obq", name="ob_q")
                for dch in range(CCH):
                    nc.scalar.add(ob[:, dch, :], pos[dch][:], bout[:, dch : dch + 1])
                nc.sync.dma_start(out=out_d[:, :, SBW : SBW + TAILW], in_=ob[:])

            # ---- main pipeline ----
            tail_block()
            for sb in range(NSB):
                a2 = dve_chunk(sb, 2, a2_pool)
                dwt = [pe_conv(sb, b, 0) for b in range(2)]
                dwt1 = [pe_conv(sb, b, 1) for b in range(2)]
                a3 = dve_chunk(sb, 3, a3_pool)
                for b in range(2):
                    lb = sb * 2 + b
                    o = b * 512
                    rhs_of = {
                        0: dwt[b][:],
                        1: dwt1[b][:],
                        2: a2[:, o : o + 512],
                        3: a3[:, o : o + 512],
                    }
                    st = (nc.sync, nc.scalar)[lb % 2] if lb < 6 else nc.gpsimd
                    pointwise(lb, rhs_of, st)

    nc.finalize()
    return nc


def _get_nc():
    if "nc" not in _cached:
        _cached["nc"] = _build_nc()
    return _cached["nc"]


def _analyze(segment_boundaries):
    starts = segment_boundaries[..., 0].astype(np.int64)  # [B,S]
    ends = segment_boundaries[..., 1].astype(np.int64)
    pos = np.arange(L)
    in_seg = (pos[None, None, :] >= starts[..., None]) & (
        pos[None, None, :] < ends[..., None]
    )  # [B,S,L]
    covered = in_seg.any(axis=1)
    seg_id = np.where(covered, in_seg.argmax(axis=1), -1)  # [B,L]
    return covered, seg_id


def kernel(x, segment_boundaries, w_dw, b_dw, w_pw, b_pw):
    from concourse.bass_utils import run_bass_kernel_spmd

    x = np.asarray(x, dtype=np.float32)
    sb = np.asarray(segment_boundaries)
    w_dw = np.asarray(w_dw, dtype=np.float32)
    b_dw = np.asarray(b_dw, dtype=np.float32)
    w_pw = np.asarray(w_pw, dtype=np.float32)
    b_pw = np.asarray(b_pw, dtype=np.float32)

    covered, seg_id = _analyze(sb)

    # ---- run decomposition + stream build ----
    pieces = []
    src_b_parts = []
    src_l_parts = []
    run_start_of = np.full((B, L), -1, np.int64)
    for b in range(B):
        sid = seg_id[b]
        change = np.nonzero(np.diff(sid) != 0)[0] + 1
        bounds = np.concatenate([[0], change, [L]])
        for s, e in zip(bounds[:-1], bounds[1:]):
            if sid[s] < 0:
                continue
            run_start_of[b, s:e] = s
            pieces.append(np.zeros((4, C), np.float32))
            src_b_parts.append(np.full(4, -1, np.int64))
            src_l_parts.append(np.full(4, -1, np.int64))
            pieces.append(x[b, s:e])
            src_b_parts.append(np.full(e - s, b, np.int64))
            src_l_parts.append(np.arange(s, e, dtype=np.int64))
    if pieces:
        stream = np.concatenate(pieces, axis=0)
        src_b = np.concatenate(src_b_parts)
        src_l = np.concatenate(src_l_parts)
    else:
        stream = np.zeros((0, C), np.float32)
        src_b = np.zeros(0, np.int64)
        src_l = np.zeros(0, np.int64)
    T = stream.shape[0]
    Q = -(-T // NCORES) if T else 1
    assert Q <= TOTW, f"stream quota {Q} too large"

    # ---- shared per-core inputs ----
    wdiag = np.ascontiguousarray(
        w_dw.reshape(CCH, 128, K).transpose(1, 0, 2).reshape(128, CCH * K)
    ).astype(np.float32)
    bias_out = w_pw @ b_dw + b_pw
    boutr = np.ascontiguousarray(bias_out.reshape(CCH, 128).T).astype(np.float32)
    cst = np.concatenate([wdiag, boutr], axis=1)
    diag = np.zeros((128, CCH * K, 128), BF16)
    idx = np.arange(128)
    diag[idx, :, idx] = wdiag.astype(BF16)[idx, :]
    wpwt = np.ascontiguousarray(
        w_pw.reshape(CCH, 128, CCH, 128).transpose(3, 2, 0, 1)
    ).astype(BF16)

    SLAB_W = 4 + TOTW + 16
    in_maps = []
    spans = []
    for i in range(NCORES):
        lo, hi = i * Q, min((i + 1) * Q, T)
        lo = min(lo, T)
        spans.append((lo, hi))
        buf = np.zeros((SLAB_W, C), np.float32)
        if hi > lo:
            hlo = max(0, lo - 4)
            buf[4 - (lo - hlo) : 4 + (hi - lo)] = stream[hlo:hi]
        slabT = np.ascontiguousarray(buf.T).astype(BF16)  # [C, SLAB_W]
        slabT = slabT.reshape(CCH, 128, SLAB_W)
        xs = np.zeros((NSB, 128, 6, XSW), BF16)
        for sbi in range(NSB):
            off = sbi * 1024
            piece = slabT[:, :, off : off + 1032]            # [CCH,128,1032]
            xs[sbi, :, 0:CCH, :1032] = piece.transpose(1, 0, 2)
            dpiece = slabT[:, :, off + 1 : off + 1033]
            xs[sbi, :, 4:6, :1032] = dpiece[2:4].transpose(1, 0, 2)
        xq = np.zeros((128, CCH, XQW), BF16)
        xq[:, :, : TAILW + 4] = slabT[:, :, SBW : SBW + TAILW + 4].transpose(1, 0, 2)
        in_maps.append({"xs": xs, "xq": xq, "cst": cst, "diag": diag, "wpwt": wpwt})

    nc = _get_nc()
    res = run_bass_kernel_spmd(nc, in_maps, list(range(NCORES)))

    # ---- gather (device out is [128, CCH, TOTW] bf16) ----
    so_out = np.zeros((T, C), np.float32)
    for i, (lo, hi) in enumerate(spans):
        if hi > lo:
            # [p, ch, t] -> [t, ch*128+p]
            full = (
                np.asarray(res.results[i]["out"])
                .astype(np.float32)
                .transpose(2, 1, 0)
                .reshape(TOTW, C)
            )
            so_out[lo:hi] = full[: hi - lo]
    out = np.zeros((B, L, C), np.float32)
    mask = src_l >= 0
    out[src_b[mask], src_l[mask]] = so_out[mask]

    # ---- general-case sparse correction (pairwise mask vs run mask) ----
    need = []
    for d in range(1, K):
        m_ref = np.zeros((B, L), bool)
        m_ref[:, d:] = covered[:, d:] & (seg_id[:, d:] == seg_id[:, :-d])
        m_run = covered & (np.arange(L)[None, :] - run_start_of >= d)
        diff = m_ref.astype(np.int8) - m_run.astype(np.int8)
        if np.any(diff):
            bs, ls = np.nonzero(diff)
            need.append((d, bs, ls, diff[bs, ls].astype(np.float32)))
    if need:
        for d, bs, ls, sgn in need:
            xv_ = x[bs, ls - d, :]
            delta_dw = xv_ * w_dw[None, :, K - 1 - d] * sgn[:, None]
            out[bs, ls, :] += delta_dw @ w_pw.T

    return out
